# revision 11
# baseline (speedup 1.0000x reference)
"""Trainium2 Bass kernel for the 2-layer GRU + 32k-vocab LM head problem.

Strategy (8 NeuronCores):
  - Vocab-shard the output projection: core c computes logits[:, :, c*4000:(c+1)*4000].
  - Replicate the (small, sequential) GRU on every core -> zero collectives.
  - Hidden-dim-as-partition "folded" layout: state tensors live in SBUF as
    (128 partitions, 8 hid-chunks x 64 batch) tiles, so gate matmuls use
    (128x128 stationary weight) x (N=64 stream), biases fold per-partition,
    and activations are wide single ops.
  - bf16 matmul inputs, fp32 PSUM accumulation, fp32 elementwise state math.
  - Phase 0: xWb[g] = W_in @ X^T + b_g  (shared layer-0 input terms, all steps).
  - Phase A: 64 sequential GRU steps (both layers), streams xWb, writes
    h1 history (bf16) and final hidden states.
  - Phase B: per step-pair, logits = [h1[t]|h1[t+1]] packed as a 128-wide
    stationary operand -> full PE utilization on the big projection.
"""

import math

import ml_dtypes
import numpy as np

EMB, HID, VOCAB, SEQ, BATCH, L = 1024, 1024, 32000, 64, 64, 2
NCORES = 8
VS = VOCAB // NCORES          # 4000 vocab per core
KC = MC = HID // 128          # 8 contraction / output chunks
FW = MC * BATCH               # 512 folded free width
VCH = 8                       # vocab chunks per pair-step
VCW = VS // VCH               # 500 (<=512, one PSUM bank)
TB = SEQ * BATCH              # 4096

_cache: dict = {}


def _build():
    import concourse.bacc as bacc
    import concourse.mybir as mybir
    import concourse.tile as tile

    dt = mybir.dt
    AF = mybir.ActivationFunctionType

    nc = bacc.Bacc("TRN2", target_bir_lowering=False, debug=False,
                   num_devices=NCORES)

    # ---- I/O ----
    xt = nc.dram_tensor("xt", [EMB, TB], dt.bfloat16, kind="ExternalInput")
    winT = nc.dram_tensor("winT", [EMB, HID], dt.bfloat16, kind="ExternalInput")
    w1T = nc.dram_tensor("w1T", [3, HID, HID], dt.bfloat16, kind="ExternalInput")
    uT = nc.dram_tensor("uT", [L, 3, HID, HID], dt.bfloat16, kind="ExternalInput")
    b0 = nc.dram_tensor("b0", [128, 24], dt.float32, kind="ExternalInput")
    b1c = nc.dram_tensor("b1c", [128, 24], dt.float32, kind="ExternalInput")
    h0i = nc.dram_tensor("h0i", [L, 128, FW], dt.float32, kind="ExternalInput")
    outwT = nc.dram_tensor("outwT", [HID, VS], dt.bfloat16, kind="ExternalInput")
    outb = nc.dram_tensor("outb", [128, VS], dt.float32, kind="ExternalInput")

    logits = nc.dram_tensor("logits", [SEQ, BATCH, VS], dt.float32,
                            kind="ExternalOutput")
    hfin = nc.dram_tensor("hfin", [L, 128, FW], dt.float32,
                          kind="ExternalOutput")

    with tile.TileContext(nc) as tc:
        with tc.tile_pool(name="dram", bufs=1, space="DRAM") as dp:
            # xwb[g][c] rows = mc*8*128 + it*128 + p (chunk c = steps 8c..8c+7)
            xwb = [[dp.tile([MC * 8 * 128, BATCH], dt.float32,
                            name=f"xwb{g}_{c}") for c in range(8)]
                   for g in range(3)]
            # h1 history rows = kc*SEQ*128 + t*128 + p, cols = batch
            h1h = dp.tile([KC * SEQ * 128, BATCH], dt.bfloat16, name="h1h")

            with tc.tile_pool(name="wA", bufs=1) as wp:
                # ---- resident weights (DMAs issue early, overlap phase 0)
                ut_sb = {}
                for l in range(L):
                    for g in range(3):
                        t_ = wp.tile([128, KC * HID], dt.bfloat16,
                                     name=f"ut{l}{g}")
                        for kc in range(KC):
                            nc.sync.dma_start(
                                out=t_[:, kc * HID:(kc + 1) * HID],
                                in_=uT[l, g, kc * 128:(kc + 1) * 128, :])
                        ut_sb[(l, g)] = t_
                w1_sb = []
                for g in range(3):
                    t_ = wp.tile([128, KC * HID], dt.bfloat16, name=f"w1{g}")
                    for kc in range(KC):
                        nc.sync.dma_start(
                            out=t_[:, kc * HID:(kc + 1) * HID],
                            in_=w1T[g, kc * 128:(kc + 1) * 128, :])
                    w1_sb.append(t_)
                b0_sb = wp.tile([128, 24], dt.float32)
                nc.sync.dma_start(out=b0_sb[:], in_=b0[:])
                b1_sb = wp.tile([128, 24], dt.float32)
                nc.sync.dma_start(out=b1_sb[:], in_=b1c[:])

                # ---- phase 0: xwb[g] = W_in @ X^T + b0[g]
                with tc.tile_pool(name="p0", bufs=1) as p0p, \
                     tc.tile_pool(name="ps0", bufs=4, space="PSUM") as pp0:
                    win_sb = p0p.tile([128, KC * HID], dt.bfloat16)
                    for kc in range(KC):
                        nc.sync.dma_start(
                            out=win_sb[:, kc * HID:(kc + 1) * HID],
                            in_=winT[kc * 128:(kc + 1) * 128, :])
                    NCH = TB // 512
                    for nch in range(NCH):
                        xts = p0p.tile([128, KC * 512], dt.bfloat16,
                                       tag="xts", bufs=2)
                        for kc in range(KC):
                            nc.sync.dma_start(
                                out=xts[:, kc * 512:(kc + 1) * 512],
                                in_=xt[kc * 128:(kc + 1) * 128,
                                       nch * 512:(nch + 1) * 512])
                        for mc in range(MC):
                            ps = pp0.tile([128, 512], dt.float32, tag="ps0")
                            for kc in range(KC):
                                nc.tensor.matmul(
                                    ps[:],
                                    win_sb[:, kc * HID + mc * 128:
                                           kc * HID + (mc + 1) * 128],
                                    xts[:, kc * 512:(kc + 1) * 512],
                                    start=(kc == 0), stop=(kc == KC - 1))
                            for g in range(3):
                                o = p0p.tile([128, 512], dt.float32,
                                             tag="p0o", bufs=4)
                                nc.vector.tensor_scalar_add(
                                    o[:], ps[:],
                                    b0_sb[:, g * 8 + mc:g * 8 + mc + 1])
                                base = mc * 8 * 128
                                nc.sync.dma_start(
                                    out=xwb[g][nch][base:base + 8 * 128, :]
                                    .rearrange("(t p) b -> p t b", p=128),
                                    in_=o[:].rearrange("p (t b) -> p t b",
                                                       t=8))

                # ---- phase A: recurrence. Layer-1 INPUT matmuls are
                # step-paired (N=128 streams halve their weight-load cost);
                # emit order software-pipelines L1 of pair p-1 against L0 of
                # pair p so the PE never starves on the recurrent dep chain.
                with tc.tile_pool(name="sA", bufs=1) as sp, \
                     tc.tile_pool(name="xw", bufs=2) as xp, \
                     tc.tile_pool(name="st", bufs=2) as stp, \
                     tc.tile_pool(name="psA", bufs=1, space="PSUM") as ppA:

                    def emit_xw(t):
                        xs = []
                        for g in range(3):
                            x_ = xp.tile([128, FW], dt.float32, tag=f"xw{g}",
                                         bufs=2, name=f"xw{g}")
                            for mc in range(MC):
                                base = mc * 8 * 128 + (t % 8) * 128
                                nc.sync.dma_start(
                                    out=x_[:, mc * BATCH:(mc + 1) * BATCH],
                                    in_=xwb[g][t // 8][base:base + 128, :])
                            xs.append(x_)
                        return xs

                    def l0_gate(g, ps_tag, rhs_bf, xw_t, func, act_tag,
                                pair_t, s):
                        ps = ppA.tile([128, FW], dt.float32, tag=ps_tag,
                                      bufs=1, name=f"ps{ps_tag}")
                        for mc in range(MC):
                            om = ps[:, mc * BATCH:(mc + 1) * BATCH]
                            for kc in range(KC):
                                nc.tensor.matmul(
                                    om,
                                    ut_sb[(0, g)][:, kc * HID + mc * 128:
                                                  kc * HID + (mc + 1) * 128],
                                    rhs_bf[:, kc * BATCH:(kc + 1) * BATCH],
                                    start=(kc == 0), stop=(kc == KC - 1))
                        pre = sp.tile([128, FW], dt.float32, tag="pre0",
                                      bufs=2, name="pre")
                        nc.vector.tensor_add(pre[:], ps[:], xw_t[:])
                        act = sp.tile([128, FW], dt.float32, tag=act_tag,
                                      bufs=1, name=f"a{act_tag}")
                        nc.scalar.activation(act[:], pre[:], func)
                        dst = pair_t[:].rearrange("p (kc sb) -> p kc sb",
                                                  kc=KC)[:, :,
                                                         s * 64:(s + 1) * 64]
                        nc.vector.tensor_copy(
                            dst, pre[:].rearrange("p (kc b) -> p kc b", kc=KC))
                        return act

                    def w1_pair(g, tag, pair_t):
                        ps = ppA.tile([128, 2 * FW], dt.float32, tag=tag,
                                      bufs=1, name=f"ps{tag}")
                        for mc in range(MC):
                            om = ps[:, mc * 128:(mc + 1) * 128]
                            for kc in range(KC):
                                # start=True clears has_written for the WHOLE
                                # bank -> only the first matmul touching each
                                # of the two banks may set it; later writes
                                # rely on per-element overwrite-then-accumulate
                                nc.tensor.matmul(
                                    om,
                                    w1_sb[g][:, kc * HID + mc * 128:
                                             kc * HID + (mc + 1) * 128],
                                    pair_t[:, kc * 128:(kc + 1) * 128],
                                    start=(kc == 0 and mc % 4 == 0),
                                    stop=False,
                                    skip_group_check=True)
                        return ps

                    def u1_pass(ps, g, s, rhs_bf):
                        for mc in range(MC):
                            om = ps[:, mc * 128 + s * 64:mc * 128 + s * 64 + 64]
                            for kc in range(KC):
                                nc.tensor.matmul(
                                    om,
                                    ut_sb[(1, g)][:, kc * HID + mc * 128:
                                                  kc * HID + (mc + 1) * 128],
                                    rhs_bf[:, kc * BATCH:(kc + 1) * BATCH],
                                    start=False, stop=(kc == KC - 1),
                                    skip_group_check=True)

                    def l1_act(ps, gi, s, func, act_tag):
                        act = sp.tile([128, FW], dt.float32, tag=act_tag,
                                      bufs=1, name=f"a{act_tag}")
                        for mc in range(MC):
                            nc.scalar.activation(
                                act[:, mc * BATCH:(mc + 1) * BATCH],
                                ps[:, mc * 128 + s * 64:
                                   mc * 128 + s * 64 + 64],
                                func,
                                bias=b1_sb[:, gi * 8 + mc:gi * 8 + mc + 1])
                        return act

                    def h_update(hf, z, th, d_tag, f_tag, b_tag):
                        d = sp.tile([128, FW], dt.float32, tag=d_tag, bufs=1,
                                    name=f"d{d_tag}")
                        nc.vector.tensor_sub(d[:], th[:], hf[:])
                        nc.vector.tensor_mul(d[:], z[:], d[:])
                        hf_n = stp.tile([128, FW], dt.float32, tag=f_tag,
                                        name=f"h{f_tag}")
                        nc.vector.tensor_add(hf_n[:], hf[:], d[:])
                        hb_n = stp.tile([128, FW], dt.bfloat16, tag=b_tag,
                                        name=f"h{b_tag}")
                        nc.vector.tensor_copy(hb_n[:], hf_n[:])
                        return hf_n, hb_n

                    def hist_dma(t, hb):
                        for kc in range(KC):
                            base = kc * SEQ * 128 + t * 128
                            nc.sync.dma_start(
                                out=h1h[base:base + 128, :],
                                in_=hb[:, kc * BATCH:(kc + 1) * BATCH])

                    def l0_step(t, s, h0f, h0b, pair):
                        xw = emit_xw(t)
                        r0 = l0_gate(0, "pl0a", h0b, xw[0], AF.Sigmoid,
                                     "r0", pair[0], s)
                        z0 = l0_gate(1, "pl0b", h0b, xw[1], AF.Sigmoid,
                                     "z0", pair[1], s)
                        rp0 = sp.tile([128, FW], dt.bfloat16, tag="rp0",
                                      bufs=2, name="rp0")
                        nc.vector.tensor_mul(rp0[:], r0[:], h0f[:])
                        th0 = l0_gate(2, "pl0a", rp0, xw[2], AF.Tanh,
                                      "th0", pair[2], s)
                        return h_update(h0f, z0, th0, "d0", "h0f", "h0b")

                    def new_pair():
                        return [sp.tile([128, 2 * FW], dt.bfloat16,
                                        tag=f"p0{g}", bufs=2, name=f"p0{g}")
                                for g in range(3)]

                    h0f = stp.tile([128, FW], dt.float32, tag="h0f")
                    nc.sync.dma_start(out=h0f[:], in_=h0i[0])
                    h0b = stp.tile([128, FW], dt.bfloat16, tag="h0b")
                    nc.vector.tensor_copy(h0b[:], h0f[:])
                    h1f = stp.tile([128, FW], dt.float32, tag="h1f")
                    nc.sync.dma_start(out=h1f[:], in_=h0i[1])
                    h1b = stp.tile([128, FW], dt.bfloat16, tag="h1b")
                    nc.vector.tensor_copy(h1b[:], h1f[:])

                    # prologue: L0 of pair 0
                    pair = new_pair()
                    h0f, h0b = l0_step(0, 0, h0f, h0b, pair)
                    h0f, h0b = l0_step(1, 1, h0f, h0b, pair)

                    for p in range(1, SEQ // 2 + 1):
                        prev, ta = pair, 2 * (p - 1)
                        last = (p == SEQ // 2)
                        if not last:
                            pair = new_pair()
                        # --- L1 of pair p-1, interleaved with L0 of pair p
                        ps1r = w1_pair(0, "ps1r", prev[0])
                        ps1f = w1_pair(1, "ps1f", prev[1])
                        u1_pass(ps1r, 0, 0, h1b)
                        u1_pass(ps1f, 1, 0, h1b)
                        r1 = l1_act(ps1r, 0, 0, AF.Sigmoid, "r1")
                        z1 = l1_act(ps1f, 1, 0, AF.Sigmoid, "z1")
                        rp1 = sp.tile([128, FW], dt.bfloat16, tag="rp1",
                                      bufs=2, name="rp1")
                        nc.vector.tensor_mul(rp1[:], r1[:], h1f[:])
                        ps1h = w1_pair(2, "ps1h", prev[2])
                        u1_pass(ps1h, 2, 0, rp1)
                        th1 = l1_act(ps1h, 2, 0, AF.Tanh, "th1")
                        h1f, h1b = h_update(h1f, z1, th1, "d1", "h1f", "h1b")
                        hist_dma(ta, h1b)
                        if not last:
                            h0f, h0b = l0_step(2 * p, 0, h0f, h0b, pair)
                        u1_pass(ps1r, 0, 1, h1b)
                        u1_pass(ps1f, 1, 1, h1b)
                        r1 = l1_act(ps1r, 0, 1, AF.Sigmoid, "r1")
                        z1 = l1_act(ps1f, 1, 1, AF.Sigmoid, "z1")
                        rp1 = sp.tile([128, FW], dt.bfloat16, tag="rp1",
                                      bufs=2, name="rp1")
                        nc.vector.tensor_mul(rp1[:], r1[:], h1f[:])
                        u1_pass(ps1h, 2, 1, rp1)
                        th1 = l1_act(ps1h, 2, 1, AF.Tanh, "th1")
                        h1f, h1b = h_update(h1f, z1, th1, "d1", "h1f", "h1b")
                        hist_dma(ta + 1, h1b)
                        if not last:
                            h0f, h0b = l0_step(2 * p + 1, 1, h0f, h0b, pair)

                    nc.sync.dma_start(out=hfin[0], in_=h0f[:])
                    nc.sync.dma_start(out=hfin[1], in_=h1f[:])

            # ================= phase B: projection =================
            with tc.tile_pool(name="wB", bufs=1) as wb, \
                 tc.tile_pool(name="sB", bufs=2) as sb2, \
                 tc.tile_pool(name="psB", bufs=1, space="PSUM") as pb:
                outw_sb = wb.tile([128, KC * VS], dt.bfloat16)
                for kc in range(KC):
                    nc.sync.dma_start(out=outw_sb[:, kc * VS:(kc + 1) * VS],
                                      in_=outwT[kc * 128:(kc + 1) * 128, :])
                outb_sb = wb.tile([128, VS], dt.float32)
                nc.sync.dma_start(out=outb_sb[:], in_=outb[:])

                for pair in range(SEQ // 2):
                    t0 = 2 * pair
                    lp = sb2.tile([128, KC * 128], dt.bfloat16, tag="lp",
                                  bufs=2)
                    for kc in range(KC):
                        base = kc * SEQ * 128 + t0 * 128
                        nc.sync.dma_start(
                            out=lp[:, kc * 128:(kc + 1) * 128].rearrange(
                                "p (s b) -> p s b", s=2),
                            in_=h1h[base:base + 256, :].rearrange(
                                "(s p) b -> p s b", p=128))
                    pst = [pb.tile([128, VCW], dt.float32, tag=f"pb{v}",
                                   bufs=1, name=f"pst{v}") for v in range(VCH)]
                    for kc in range(KC):
                        for vc in range(VCH):
                            nc.tensor.matmul(
                                pst[vc][:],
                                lp[:, kc * 128:(kc + 1) * 128],
                                outw_sb[:, kc * VS + vc * VCW:
                                        kc * VS + (vc + 1) * VCW],
                                start=(kc == 0), stop=(kc == KC - 1))
                    for vc in range(VCH):
                        ob = sb2.tile([128, VCW], dt.float32, tag="ob", bufs=8)
                        nc.vector.tensor_add(
                            ob[:], pst[vc][:],
                            outb_sb[:, vc * VCW:(vc + 1) * VCW])
                        for s in range(2):
                            nc.sync.dma_start(
                                out=logits[t0 + s, :, vc * VCW:(vc + 1) * VCW],
                                in_=ob[s * 64:(s + 1) * 64, :])

    nc.compile()
    return nc


def _fold_state(h):
    # (L, BATCH, HID) -> (L, 128, FW) with [l, p, c*64+b] = h[l, b, c*128+p]
    return np.ascontiguousarray(
        h.reshape(L, BATCH, MC, 128).transpose(0, 3, 2, 1).reshape(L, 128, FW)
    ).astype(np.float32)


def kernel(inputs, hidden, emb_table, W_in, Wr1, Wf1, Wh1,
           Ur, br, Uf, bf, Uh, bh, out_W, out_b):
    inputs, hidden, emb_table = (np.asarray(x) for x in
                                 (inputs, hidden, emb_table))
    W_in, Wr1, Wf1, Wh1 = (np.asarray(x, np.float32) for x in
                           (W_in, Wr1, Wf1, Wh1))
    Ur, br, Uf, bf, Uh, bh = (np.asarray(x, np.float32) for x in
                              (Ur, br, Uf, bf, Uh, bh))
    out_W, out_b = np.asarray(out_W, np.float32), np.asarray(out_b, np.float32)
    bf16 = ml_dtypes.bfloat16
    if "nc" not in _cache:
        _cache["nc"] = _build()
    nc = _cache["nc"]
    from concourse.bass_utils import run_bass_kernel_spmd

    sqrt_e = np.float32(math.sqrt(EMB))
    X = (emb_table[inputs.astype(np.int64)] * sqrt_e).astype(np.float32)
    xt_h = np.ascontiguousarray(X.reshape(TB, EMB).T).astype(bf16)
    winT_h = np.ascontiguousarray(W_in.T).astype(bf16)
    w1T_h = np.stack([np.ascontiguousarray(w.T)
                      for w in (Wr1, Wf1, Wh1)]).astype(bf16)
    uT_h = np.stack([
        np.stack([np.ascontiguousarray(U[l].T) for U in (Ur, Uf, Uh)])
        for l in range(L)]).astype(bf16)

    b0_h = np.zeros((128, 24), np.float32)
    b1_h = np.zeros((128, 24), np.float32)
    for g, b_ in enumerate((br, bf, bh)):
        b0_h[:, g * 8:(g + 1) * 8] = b_[0].reshape(MC, 128).T
        b1_h[:, g * 8:(g + 1) * 8] = b_[1].reshape(MC, 128).T
    h0i_h = _fold_state(np.asarray(hidden))

    base = dict(xt=xt_h, winT=winT_h, w1T=w1T_h, uT=uT_h, b0=b0_h, b1c=b1_h,
                h0i=h0i_h)
    in_maps = []
    for c in range(NCORES):
        sl = slice(c * VS, (c + 1) * VS)
        in_maps.append(dict(
            base,
            outwT=np.ascontiguousarray(out_W[sl].T).astype(bf16),
            outb=np.ascontiguousarray(
                np.broadcast_to(out_b[sl], (128, VS))).astype(np.float32),
        ))

    res = run_bass_kernel_spmd(nc, in_maps, list(range(NCORES)))

    logits_full = np.empty((SEQ, BATCH, VOCAB), np.float32)
    for c in range(NCORES):
        logits_full[:, :, c * VS:(c + 1) * VS] = res.results[c]["logits"]
    hf = res.results[0]["hfin"]  # (L, 128, FW)
    h_final = np.ascontiguousarray(
        hf.reshape(L, 128, MC, BATCH).transpose(0, 3, 2, 1)
        .reshape(L * BATCH, HID))
    return logits_full, h_final
